# revision 21
# baseline (speedup 1.0000x reference)
"""Trainium2 Bass kernel for a 2-layer GAT (4 heads, 32 dim/head) + linear classifier.

v2: fp16 edge pipeline. Same 3-launch structure as v1 (host does the
edge gather between launches; all FLOPs on device), but:

- All edge-path tensors are fp16: EA split into contiguous per-partition
  sections (eah [P, TOT*128] h-rows, eas/ead [P, TOT*4] attention cols)
  so every DMA lands as large contiguous per-partition chunks.
- All matmuls run in fp16 (fp32 matmul is 4x slower on TRN2 PE).
- Banded one-hot: slots are band-aligned (band width W=32, 4 bands per
  128-node block; per-(block,band) tile counts are max'd over cores so
  the program stays SPMD). The is_equal one-hot is [P, T*32] instead of
  [P, T*128], and each aggregation matmul writes a 32-partition slice of
  PSUM via PE tile_position.
- DVE 2x packing everywhere it matters: w is exp'd into pair-replicated
  form (w2), meta is host-built pair-replicated (m2), so the big h*w
  multiply and the one-hot compare run at 2 elem/cycle/lane.
- Small ops are spread across ACT (lrelu/exp/relu/copies) and GPSIMD
  (adds) to keep DVE, the bottleneck engine, lean.
- exp uses a constant -4 bias (softmax-invariant) so fp16 w can't
  overflow.
"""

import os
import sys
import time

for _p in ("/opt/trn_rl_repo", "/root/.axon_site/_ro/trn_rl_repo"):
    if os.path.isdir(_p) and _p not in sys.path:
        sys.path.insert(0, _p)

import dataclasses

import numpy as np

import concourse.bass as bass
import concourse.mybir as mybir
import concourse.tile as tile
from concourse import bacc
from concourse.bass_utils import run_bass_kernel_spmd

P = 128
D = 128
HEADS = 4
C = 40
NEG_SLOPE = 0.2
ROW = D + 2 * HEADS  # 136: [h | a_src | a_dst] in nrows/zrows
RW = D + HEADS  # 132: matmul rhs [h*w | w]
W = 64  # band width (PSUM base partition must be in {0, 32, 64})
NBAND = P // W
EXP_BIAS = -4.0  # softmax-invariant shift to keep fp16 exp in range
NCORES = 8

f32 = mybir.dt.float32
f16 = mybir.dt.float16

LAST_INFO = {}  # stash for test.py / dev harnesses

# A/B knobs (all measured on HW):
#   lrelu: "act" (1 ACT Lrelu) | "dve" (2 DVE ops)
#   ew_eng / hadd_eng: "gp" | "dve"
#   div: "div" (ACT s-copy + DVE divide) | "rcp" (DVE s_eps+rcp+mult)
#   trans: "dma" (dma_start_transpose) | "pe" (PE transpose via ident)
#   start_mode: "t0" | "per_band" (PSUM has_written semantics probe)
CONFIG = {
    "lrelu": "prelu",  # ACT Prelu honors alpha (Lrelu is off by 4x on this HW)
    "noeps": True,  # rcp directly on s (s>0 for all real rows; pad rows discarded)
    "gw_gp_frac": 0.10,  # fraction of gw tiles multiplied on GPSIMD instead of DVE
    "ew_eng": "gp",
    "hadd_eng": "gp",
    "div": "rcp",  # "div" rejected: DVE TT divide fails s3s3d3_tt_valid_op ISA check
    "trans": "dma",
    "start_mode": "per_band",  # PE start=True clears only its own PSUM region
    "wbufs": 3,
    "gbufs": 3,
}


def _ap_with(ap, dims):
    return dataclasses.replace(ap, ap=dims)


def _free_dims(ap, dims):
    """Replace the free dims of `ap` (keep partition dim ap[0])."""
    return _ap_with(ap, [ap.ap[0]] + dims)


def build_node_transform(nper, wcols, repeat=1, trn_type="TRN2"):
    """Launch A: nrows[n] = x[n] @ wcat, all fp16 (psum accum f32)."""
    nc = bacc.Bacc(trn_type, target_bir_lowering=False, debug=False, num_devices=NCORES)
    xts_d = nc.dram_tensor("xts", [P, nper], f16, kind="ExternalInput")
    wcat_d = nc.dram_tensor("wcat", [D, wcols], f16, kind="ExternalInput")
    out_d = nc.dram_tensor("nrows", [nper, wcols], f16, kind="ExternalOutput")
    nt = nper // P
    with tile.TileContext(nc) as tc:
        with (
            tc.tile_pool(name="const", bufs=1) as cpool,
            tc.tile_pool(name="work", bufs=4) as pool,
            tc.tile_pool(name="psum", bufs=4, space="PSUM") as psum,
        ):
            wcat = cpool.tile([D, wcols], f16, tag="wcat")
            nc.sync.dma_start(wcat[:], wcat_d[:])
            for rep in range(repeat):
                if rep:
                    tc.strict_bb_all_engine_barrier()
                for i in range(nt):
                    xt = pool.tile([P, P], f16, tag="xt")
                    nc.sync.dma_start(xt[:], xts_d[:, i * P : (i + 1) * P])
                    hp = psum.tile([P, wcols], f32, tag="hp")
                    nc.tensor.matmul(hp[:], lhsT=xt[:], rhs=wcat[:], start=True, stop=True)
                    hf = pool.tile([P, wcols], f16, tag="hf")
                    nc.scalar.copy(hf[:], hp[:])
                    nc.sync.dma_start(out_d[i * P : (i + 1) * P, :], hf[:])
    nc.compile()
    return nc


def build_gat_layer(nper, Ts, bandof, wcols, is_last, repeat=1, trn_type="TRN2", cfg=None):
    """Launch B/C: banded edge aggregation + node phase for one GAT layer.

    Ts: per-block tile counts (static, same all cores). bandof: per-block
    list of per-tile band indices. Inputs eah/eas/ead/m2 are slot-major
    per-partition contiguous fp16.
    """
    cfg = dict(CONFIG if cfg is None else cfg)
    nblocks = nper // P
    assert len(Ts) == nblocks
    TOT = int(sum(Ts))
    offs = np.concatenate([[0], np.cumsum(Ts)]).astype(int)
    zdt = f32 if is_last else f16

    nc = bacc.Bacc(trn_type, target_bir_lowering=False, debug=False, num_devices=NCORES)
    eah_d = nc.dram_tensor("eah", [P, TOT * D], f16, kind="ExternalInput")
    eas_d = nc.dram_tensor("eas", [P, TOT * HEADS], f16, kind="ExternalInput")
    ead_d = nc.dram_tensor("ead", [P, TOT * HEADS], f16, kind="ExternalInput")
    m2_d = nc.dram_tensor("m2", [P, TOT * 2], f16, kind="ExternalInput")
    wnext_d = nc.dram_tensor("wnext", [D, wcols], f16, kind="ExternalInput")
    brep_d = nc.dram_tensor("brep", [P, D], f16, kind="ExternalInput")
    bout_d = nc.dram_tensor("bout", [P, wcols], f32, kind="ExternalInput")
    iota_d = nc.dram_tensor("iota", [P, W], f16, kind="ExternalInput")
    ident_d = nc.dram_tensor("ident", [P, P], f16, kind="ExternalInput")
    out_d = nc.dram_tensor("zrows", [nper, wcols], zdt, kind="ExternalOutput")
    dbg = bool(cfg.get("debug"))
    if dbg:
        dbg_ew = nc.dram_tensor("dbg_ew", [P, TOT * HEADS], f16, kind="ExternalOutput")
        dbg_w2 = nc.dram_tensor("dbg_w2", [P, TOT * HEADS * 2], f16, kind="ExternalOutput")
        dbg_gw = nc.dram_tensor("dbg_gw", [P, TOT * RW], f16, kind="ExternalOutput")
        dbg_a01 = nc.dram_tensor("dbg_a01", [P, TOT * W], f16, kind="ExternalOutput")
        dbg_u = nc.dram_tensor("dbg_u", [P, nblocks * RW], f32, kind="ExternalOutput")
        dbg_h = nc.dram_tensor("dbg_h", [P, nblocks * D], f16, kind="ExternalOutput")
        dbg_ht = nc.dram_tensor("dbg_ht", [P, nblocks * P], f16, kind="ExternalOutput")

    with tile.TileContext(nc) as tc:
        with (
            tc.tile_pool(name="const", bufs=1) as cpool,
            tc.tile_pool(name="work", bufs=int(cfg.get("wbufs", 3))) as pool,
            tc.tile_pool(name="gath", bufs=int(cfg.get("gbufs", 3))) as gpool,
            tc.tile_pool(name="psum", bufs=2, space="PSUM") as psum,
            tc.tile_pool(name="psU", bufs=2, space="PSUM") as psU,
        ):
            wnext = cpool.tile([D, wcols], f16, tag="wnext")
            nc.sync.dma_start(wnext[:], wnext_d[:])
            brep = cpool.tile([P, D], f16, tag="brep")
            nc.sync.dma_start(brep[:], brep_d[:])
            bout = cpool.tile([P, wcols], f32, tag="bout")
            nc.sync.dma_start(bout[:], bout_d[:])
            iota = cpool.tile([P, W], f16, tag="iota")
            nc.sync.dma_start(iota[:], iota_d[:])
            ident = cpool.tile([P, P], f16, tag="ident")
            nc.sync.dma_start(ident[:], ident_d[:])
            m2all = cpool.tile([P, TOT * 2], f16, tag="m2all")
            nc.sync.dma_start(m2all[:], m2_d[:])
            ebias = cpool.tile([P, 1], f32, tag="ebias")
            nc.vector.memset(ebias[:], EXP_BIAS)

            for rep in range(repeat):
                if rep:
                    tc.strict_bb_all_engine_barrier()
                for b in range(nblocks):
                    T = int(Ts[b])
                    if T == 0:
                        continue
                    off = int(offs[b])
                    bands = bandof[b]

                    Gh = gpool.tile([P, T * D], f16, tag="Gh")
                    nc.sync.dma_start(Gh[:], eah_d[:, off * D : (off + T) * D])
                    Ga = pool.tile([P, T * HEADS], f16, tag="Ga")
                    nc.sync.dma_start(Ga[:], eas_d[:, off * HEADS : (off + T) * HEADS])
                    Gd = pool.tile([P, T * HEADS], f16, tag="Gd")
                    nc.sync.dma_start(Gd[:], ead_d[:, off * HEADS : (off + T) * HEADS])

                    # e = a_src + a_dst ; leaky
                    ew = pool.tile([P, T * HEADS], f16, tag="ew")
                    ew_eng = nc.gpsimd if cfg["ew_eng"] == "gp" else nc.vector
                    ew_eng.tensor_tensor(out=ew[:], in0=Ga[:], in1=Gd[:], op=mybir.AluOpType.add)
                    lrelu = cfg["lrelu"]
                    if lrelu in ("act", "act8"):
                        nc.scalar.activation(ew[:], ew[:], mybir.ActivationFunctionType.Lrelu,
                                             alpha=NEG_SLOPE * (4.0 if lrelu == "act8" else 1.0))
                    elif lrelu in ("prelu", "prelu8"):
                        nc.scalar.activation(ew[:], ew[:], mybir.ActivationFunctionType.Prelu,
                                             alpha=NEG_SLOPE * (4.0 if lrelu == "prelu8" else 1.0))
                    else:
                        # "mixed": scale on GPSIMD, max on DVE (Pool lacks max)
                        lr_eng = nc.gpsimd if lrelu == "mixed" else nc.vector
                        lk = pool.tile([P, T * HEADS], f16, tag="lk")
                        lr_eng.tensor_scalar(lk[:], ew[:], NEG_SLOPE, None, mybir.AluOpType.mult)
                        nc.vector.tensor_tensor(out=ew[:], in0=ew[:], in1=lk[:], op=mybir.AluOpType.max)

                    # w2[p, t, head, 0:2] = exp(e - 4)  (pair-replicated)
                    w2 = pool.tile([P, T * HEADS * 2], f16, tag="w2")
                    ew_b = _free_dims(ew[:], [[HEADS, T], [1, HEADS], [0, 2]])
                    w2_o = _free_dims(w2[:], [[2 * HEADS, T], [2, HEADS], [1, 2]])
                    nc.scalar.activation(w2_o, ew_b, mybir.ActivationFunctionType.Exp,
                                         bias=ebias[:])

                    # GW = [h*w | w] per tile (RW=132 cols)
                    GW = gpool.tile([P, T * RW], f16, tag="GW")
                    gw3 = GW[:].rearrange("p (t c) -> p t c", c=RW)
                    # w cols: exp again, strided out
                    nc.scalar.activation(
                        gw3[:, :, D:RW],
                        _free_dims(ew[:], [[HEADS, T], [1, HEADS]]),
                        mybir.ActivationFunctionType.Exp,
                        bias=ebias[:],
                    )
                    # h*w: DVE 2x (all APs innermost step-1 fp16); optionally a
                    # leading fraction of tiles goes to GPSIMD instead
                    kgp = int(round(T * float(cfg.get("gw_gp_frac", 0.0))))
                    PAIR_REP = D // HEADS // 2
                    if kgp > 0:
                        nc.gpsimd.tensor_tensor(
                            out=gw3[:, 0:kgp, 0:D],
                            in0=_free_dims(Gh[:], [[D, kgp], [1, D]]),
                            in1=_free_dims(w2[:], [[2 * HEADS, kgp], [2, HEADS], [0, PAIR_REP], [1, 2]]),
                            op=mybir.AluOpType.mult,
                        )
                    if kgp < T:
                        w2k = w2[:, kgp * 2 * HEADS : T * 2 * HEADS]
                        ghk = Gh[:, kgp * D : T * D]
                        nc.vector.tensor_tensor(
                            out=gw3[:, kgp:T, 0:D],
                            in0=_free_dims(ghk, [[D, T - kgp], [1, D]]),
                            in1=_free_dims(w2k, [[2 * HEADS, T - kgp], [2, HEADS], [0, PAIR_REP], [1, 2]]),
                            op=mybir.AluOpType.mult,
                        )

                    # banded one-hot A01[p, t, d] = (m2[p,t] == d), d in [0,32)
                    A01 = gpool.tile([P, T * W], f16, tag="A01")
                    m2_in = _free_dims(m2all[:], [[2, T], [0, W // 2], [1, 2]])
                    m2_in = dataclasses.replace(m2_in, offset=m2_in.offset + 2 * off)
                    iota_in = _free_dims(iota[:], [[0, T], [1, W]])
                    nc.vector.tensor_tensor(
                        out=A01[:], in0=m2_in, in1=iota_in, op=mybir.AluOpType.is_equal
                    )

                    # aggregate: U[band*W:(band+1)*W] += A01_t.T @ [h*w | w]_t
                    a3 = A01[:].rearrange("p (t d) -> p t d", d=W)
                    U = psU.tile([P, RW], f32, tag="U")
                    if cfg["start_mode"] == "t0":
                        starts = [t == 0 for t in range(T)]
                    else:
                        seen = set()
                        starts = []
                        for t in range(T):
                            starts.append(bands[t] not in seen)
                            seen.add(bands[t])
                    for t in range(T):
                        Bt = int(bands[t]) * W
                        nc.tensor.matmul(
                            U[Bt : Bt + W, :],
                            lhsT=a3[:, t, :],
                            rhs=gw3[:, t, :],
                            start=starts[t],
                            stop=(t == T - 1),
                            skip_group_check=True,
                        )

                    if dbg:
                        nc.sync.dma_start(dbg_ew[:, off * HEADS : (off + T) * HEADS], ew[:])
                        nc.sync.dma_start(dbg_w2[:, off * HEADS * 2 : (off + T) * HEADS * 2], w2[:])
                        nc.sync.dma_start(dbg_gw[:, off * RW : (off + T) * RW], GW[:])
                        nc.sync.dma_start(dbg_a01[:, off * W : (off + T) * W], A01[:])
                        ucp = pool.tile([P, RW], f32, tag="ucp")
                        nc.scalar.copy(ucp[:], U[:, :])
                        nc.sync.dma_start(dbg_u[:, b * RW : (b + 1) * RW], ucp[:])

                    # node phase: h = relu(U_h / s + b)
                    h = pool.tile([P, D], f16, tag="h")
                    if cfg["div"] == "div":
                        s_sb = pool.tile([P, HEADS], f32, tag="s_sb")
                        nc.scalar.copy(s_sb[:], U[:, D:RW])
                        s_b = _free_dims(s_sb[:], [[1, HEADS], [0, D // HEADS]])
                        nc.vector.tensor_tensor(
                            out=h[:], in0=U[:, 0:D], in1=s_b, op=mybir.AluOpType.divide
                        )
                    else:
                        rcp = pool.tile([P, HEADS], f32, tag="rcp")
                        if cfg.get("noeps"):
                            nc.vector.reciprocal(rcp[:], U[:, D:RW])
                        else:
                            s_eps = pool.tile([P, HEADS], f32, tag="s_eps")
                            nc.vector.tensor_scalar_add(s_eps[:], U[:, D:RW], 1e-16)
                            nc.vector.reciprocal(rcp[:], s_eps[:])
                        rcp_b = _free_dims(rcp[:], [[1, HEADS], [0, D // HEADS]])
                        nc.vector.tensor_tensor(
                            out=h[:], in0=U[:, 0:D], in1=rcp_b, op=mybir.AluOpType.mult
                        )
                    hadd_eng = nc.gpsimd if cfg["hadd_eng"] == "gp" else nc.vector
                    hadd_eng.tensor_tensor(out=h[:], in0=h[:], in1=brep[:], op=mybir.AluOpType.add)
                    nc.scalar.activation(h[:], h[:], mybir.ActivationFunctionType.Relu)

                    # project: zrows = hT.T @ wnext (+ bout for last layer)
                    hT = pool.tile([P, P], f16, tag="hT")
                    if cfg["trans"] == "dma":
                        nc.sync.dma_start_transpose(hT[:], h[:])
                    else:
                        hTp = psum.tile([P, P], f16, tag="hTp")
                        nc.tensor.transpose(hTp[:], h[:], ident[:])
                        nc.scalar.copy(hT[:], hTp[:])
                    if dbg:
                        nc.sync.dma_start(dbg_h[:, b * D : (b + 1) * D], h[:])
                        nc.sync.dma_start(dbg_ht[:, b * P : (b + 1) * P], hT[:])
                    zp = psum.tile([P, wcols], f32, tag="zp")
                    nc.tensor.matmul(zp[:], lhsT=hT[:], rhs=wnext[:], start=True, stop=True)
                    z = pool.tile([P, wcols], zdt, tag="z")
                    if is_last:
                        nc.vector.tensor_tensor(out=z[:], in0=zp[:], in1=bout[:], op=mybir.AluOpType.add)
                    else:
                        nc.scalar.copy(z[:], zp[:])
                    nc.sync.dma_start(out_d[b * P : (b + 1) * P, :], z[:])

    nc.compile()
    return nc


def prep_edges(edge_index, n, ncores):
    """Band-aligned slot layout. Returns dict with static structure
    (Ts, bandof, offs) and per-core slot data (ss, ds, m2)."""
    nper = -(-n // (ncores * P)) * P
    npad = nper * ncores
    nblocks = nper // P

    e0 = np.asarray(edge_index[0], dtype=np.int64)
    e1 = np.asarray(edge_index[1], dtype=np.int64)
    loops = np.arange(n, dtype=np.int64)
    src = np.concatenate([e0, loops])
    dst = np.concatenate([e1, loops])
    order = np.argsort(dst, kind="stable")
    srcs = src[order]
    dsts = dst[order]

    # per (core, block, band) counts via searchsorted on band boundaries
    bounds = np.searchsorted(dsts, np.arange(0, npad + W, W))  # [npad/W + 1]
    cnt = (bounds[1:] - bounds[:-1]).reshape(ncores, nblocks, NBAND)
    tiles = -(-cnt.max(axis=0) // P)  # [nblocks, NBAND] (0 where empty)
    Ts = tiles.sum(axis=1)  # [nblocks]
    TOT = int(Ts.sum())
    offs = np.concatenate([[0], np.cumsum(Ts)]).astype(int)
    bandof = [
        np.repeat(np.arange(NBAND), tiles[b]).astype(int).tolist()
        for b in range(nblocks)
    ]
    # tile start (within block) of each band
    tbase = np.concatenate(
        [np.zeros((nblocks, 1), int), np.cumsum(tiles, axis=1)[:, :-1]], axis=1
    )

    ss_l, ds_l, m2_l = [], [], []
    for c in range(ncores):
        ss = np.zeros(TOT * P, dtype=np.int64)
        ds_ = np.zeros(TOT * P, dtype=np.int64)
        meta = np.full((P, TOT), 300.0, dtype=np.float16)
        for b in range(nblocks):
            for j in range(NBAND):
                g = (c * nblocks + b) * NBAND + j
                lo, hi = int(bounds[g]), int(bounds[g + 1])
                cntb = hi - lo
                if cntb == 0:
                    continue
                t0 = int(offs[b] + tbase[b, j])
                k = np.arange(cntb)
                slot = (t0 + k // P) * P + (k % P)
                ss[slot] = srcs[lo:hi]
                ds_[slot] = dsts[lo:hi]
                loc = (dsts[lo:hi] - (c * nper + b * P + j * W)).astype(np.float16)
                meta[k % P, t0 + k // P] = loc
        m2 = np.repeat(meta, 2, axis=1)  # [P, TOT*2] pair-replicated
        ss_l.append(ss)
        ds_l.append(ds_)
        m2_l.append(np.ascontiguousarray(m2))
    return {
        "Ts": Ts, "bandof": bandof, "offs": offs, "TOT": TOT, "npad": npad,
        "ss": ss_l, "ds": ds_l, "m2": m2_l,
    }


def expand_rows(nrows_full, prep):
    """Per-core (eah, eas, ead) in slot-major per-partition-contiguous
    fp16 layout. nrows_full: [npad, ROW] fp16."""
    TOT = prep["TOT"]
    outs = []
    for ss, ds_ in zip(prep["ss"], prep["ds"]):
        g = nrows_full[ss]  # [TOT*P, ROW]
        eah = np.ascontiguousarray(
            g[:, 0:D].reshape(TOT, P, D).transpose(1, 0, 2).reshape(P, TOT * D)
        )
        eas = np.ascontiguousarray(
            g[:, D : D + HEADS].reshape(TOT, P, HEADS).transpose(1, 0, 2).reshape(P, TOT * HEADS)
        )
        gd = nrows_full[ds_, D + HEADS : ROW]
        ead = np.ascontiguousarray(
            gd.reshape(TOT, P, HEADS).transpose(1, 0, 2).reshape(P, TOT * HEADS)
        )
        outs.append((eah, eas, ead))
    return outs


def amat(att):
    A = np.zeros((D, HEADS), dtype=np.float32)
    att = np.asarray(att, dtype=np.float32)
    for h in range(HEADS):
        A[h * (D // HEADS) : (h + 1) * (D // HEADS), h] = att[h]
    return A


_cache = {}


def run_gat(x, edge_index, W1, att_src1, att_dst1, b1, W2, att_src2, att_dst2, b2,
            Wc, bc, n=None, ncores=NCORES, repeat=1, cfg=None):
    global LAST_INFO
    x = np.asarray(x, dtype=np.float32)
    if n is None:
        n = int(x.shape[0])

    t0 = time.time()
    prep = prep_edges(edge_index, n, ncores)
    npad = prep["npad"]
    nper = npad // ncores
    cfg = dict(CONFIG if cfg is None else cfg)
    key = (npad, tuple(prep["Ts"]), ncores, repeat, tuple(sorted(cfg.items())))
    t1 = time.time()
    if key in _cache:
        ncA, ncB, ncC = _cache[key]
    else:
        ncA = build_node_transform(nper, ROW)
        ncB = build_gat_layer(nper, prep["Ts"], prep["bandof"], ROW, is_last=False,
                              repeat=repeat, cfg=cfg)
        ncC = build_gat_layer(nper, prep["Ts"], prep["bandof"], C, is_last=True,
                              repeat=repeat, cfg=cfg)
        _cache[key] = (ncA, ncB, ncC)
    t2 = time.time()

    W1 = np.asarray(W1, dtype=np.float32)
    W2 = np.asarray(W2, dtype=np.float32)
    Wc = np.asarray(Wc, dtype=np.float32)
    w1cat = np.concatenate([W1, W1 @ amat(att_src1), W1 @ amat(att_dst1)], axis=1).astype(np.float16)
    w2cat = np.concatenate([W2, W2 @ amat(att_src2), W2 @ amat(att_dst2)], axis=1).astype(np.float16)
    b1r = np.tile(np.asarray(b1, np.float16)[None, :], (P, 1))
    b2r = np.tile(np.asarray(b2, np.float16)[None, :], (P, 1))
    bc = np.asarray(bc, dtype=np.float32)
    iota = np.tile(np.arange(W, dtype=np.float16), (P, 1))
    ident = np.eye(P, dtype=np.float16)
    zero_bout = np.zeros((P, ROW), np.float32)

    xp = np.zeros((npad, D), dtype=np.float16)
    xp[:n] = x.astype(np.float16)

    # Launch A
    mapsA = [
        {"xts": np.ascontiguousarray(xp[c * nper : (c + 1) * nper].T), "wcat": w1cat}
        for c in range(ncores)
    ]
    resA = run_bass_kernel_spmd(ncA, mapsA, list(range(ncores)))
    nrows_full = np.concatenate([resA.results[c]["nrows"] for c in range(ncores)], axis=0)
    t3 = time.time()

    # Expansion 1 + Launch B
    ea1 = expand_rows(nrows_full, prep)
    mapsB = [
        {"eah": ea1[c][0], "eas": ea1[c][1], "ead": ea1[c][2], "m2": prep["m2"][c],
         "wnext": w2cat, "brep": b1r, "bout": zero_bout, "iota": iota, "ident": ident}
        for c in range(ncores)
    ]
    resB = run_bass_kernel_spmd(ncB, mapsB, list(range(ncores)))
    zrows_full = np.concatenate([resB.results[c]["zrows"] for c in range(ncores)], axis=0)
    t4 = time.time()

    # Expansion 2 + Launch C
    ea2 = expand_rows(zrows_full, prep)
    mapsC = [
        {"eah": ea2[c][0], "eas": ea2[c][1], "ead": ea2[c][2], "m2": prep["m2"][c],
         "wnext": Wc.astype(np.float16), "brep": b2r,
         "bout": np.tile(bc[None, :], (P, 1)), "iota": iota, "ident": ident}
        for c in range(ncores)
    ]
    resC = run_bass_kernel_spmd(ncC, mapsC, list(range(ncores)))
    out = np.concatenate([resC.results[c]["zrows"] for c in range(ncores)], axis=0)[:n]
    t5 = time.time()

    LAST_INFO = {
        "prep_s": t1 - t0, "build_s": t2 - t1, "launchA_s": t3 - t2,
        "launchB_s": t4 - t3, "launchC_s": t5 - t4,
        "ncs": (ncA, ncB, ncC),
        "maps": (mapsA, mapsB, mapsC),
        "prep": prep,
    }
    print(
        f"[kernel] prep={t1 - t0:.2f}s build={t2 - t1:.2f}s A={t3 - t2:.2f}s "
        f"B={t4 - t3:.2f}s C={t5 - t4:.2f}s",
        file=sys.stderr,
    )
    return out.astype(np.float32)


def kernel(x, edge_index, W1, att_src1, att_dst1, b1, W2, att_src2, att_dst2, b2, Wc, bc):
    return run_gat(x, edge_index, W1, att_src1, att_dst1, b1,
                   W2, att_src2, att_dst2, b2, Wc, bc)


# revision 24
# speedup vs baseline: 1.0608x; 1.0608x over previous
"""Trainium2 Bass kernel for a 2-layer GAT (4 heads, 32 dim/head) + linear classifier.

v2: fp16 edge pipeline. Same 3-launch structure as v1 (host does the
edge gather between launches; all FLOPs on device), but:

- All edge-path tensors are fp16: EA split into contiguous per-partition
  sections (eah [P, TOT*128] h-rows, eas/ead [P, TOT*4] attention cols)
  so every DMA lands as large contiguous per-partition chunks.
- All matmuls run in fp16 (fp32 matmul is 4x slower on TRN2 PE).
- Banded one-hot: slots are band-aligned (band width W=32, 4 bands per
  128-node block; per-(block,band) tile counts are max'd over cores so
  the program stays SPMD). The is_equal one-hot is [P, T*32] instead of
  [P, T*128], and each aggregation matmul writes a 32-partition slice of
  PSUM via PE tile_position.
- DVE 2x packing everywhere it matters: w is exp'd into pair-replicated
  form (w2), meta is host-built pair-replicated (m2), so the big h*w
  multiply and the one-hot compare run at 2 elem/cycle/lane.
- Small ops are spread across ACT (lrelu/exp/relu/copies) and GPSIMD
  (adds) to keep DVE, the bottleneck engine, lean.
- exp uses a constant -4 bias (softmax-invariant) so fp16 w can't
  overflow.
"""

import os
import sys
import time

for _p in ("/opt/trn_rl_repo", "/root/.axon_site/_ro/trn_rl_repo"):
    if os.path.isdir(_p) and _p not in sys.path:
        sys.path.insert(0, _p)

import dataclasses

import numpy as np

import concourse.bass as bass
import concourse.mybir as mybir
import concourse.tile as tile
from concourse import bacc
from concourse.bass_utils import run_bass_kernel_spmd

P = 128
D = 128
HEADS = 4
C = 40
NEG_SLOPE = 0.2
ROW = D + 2 * HEADS  # 136: [h | a_src | a_dst] in nrows/zrows
RW = D + HEADS  # 132: matmul rhs [h*w | w]
W = 64  # band width (PSUM base partition must be in {0, 32, 64})
NBAND = P // W
EXP_BIAS = -4.0  # softmax-invariant shift to keep fp16 exp in range
NCORES = 8

f32 = mybir.dt.float32
f16 = mybir.dt.float16

LAST_INFO = {}  # stash for test.py / dev harnesses

# A/B knobs (all measured on HW):
#   lrelu: "act" (1 ACT Lrelu) | "dve" (2 DVE ops)
#   ew_eng / hadd_eng: "gp" | "dve"
#   div: "div" (ACT s-copy + DVE divide) | "rcp" (DVE s_eps+rcp+mult)
#   trans: "dma" (dma_start_transpose) | "pe" (PE transpose via ident)
#   start_mode: "t0" | "per_band" (PSUM has_written semantics probe)
CONFIG = {
    "lrelu": "prelu",  # ACT Prelu honors alpha (Lrelu is off by 4x on this HW)
    "noeps": True,  # rcp directly on s (s>0 for all real rows; pad rows discarded)
    "gw_gp_frac": 0.15,  # fraction of gw tiles multiplied on GPSIMD instead of DVE
    "ew_eng": "gp",
    "hadd_eng": "gp",
    "div": "rcp",  # "div" rejected: DVE TT divide fails s3s3d3_tt_valid_op ISA check
    "trans": "dma",
    "start_mode": "per_band",  # PE start=True clears only its own PSUM region
    "wbufs": 4,
    "gbufs": 4,
    "ubufs": 3,
    "zbufs": 3,
}


def _ap_with(ap, dims):
    return dataclasses.replace(ap, ap=dims)


def _free_dims(ap, dims):
    """Replace the free dims of `ap` (keep partition dim ap[0])."""
    return _ap_with(ap, [ap.ap[0]] + dims)


def build_node_transform(nper, wcols, repeat=1, trn_type="TRN2"):
    """Launch A: nrows[n] = x[n] @ wcat, all fp16 (psum accum f32)."""
    nc = bacc.Bacc(trn_type, target_bir_lowering=False, debug=False, num_devices=NCORES)
    xts_d = nc.dram_tensor("xts", [P, nper], f16, kind="ExternalInput")
    wcat_d = nc.dram_tensor("wcat", [D, wcols], f16, kind="ExternalInput")
    out_d = nc.dram_tensor("nrows", [nper, wcols], f16, kind="ExternalOutput")
    nt = nper // P
    with tile.TileContext(nc) as tc:
        with (
            tc.tile_pool(name="const", bufs=1) as cpool,
            tc.tile_pool(name="work", bufs=4) as pool,
            tc.tile_pool(name="psum", bufs=4, space="PSUM") as psum,
        ):
            wcat = cpool.tile([D, wcols], f16, tag="wcat")
            nc.sync.dma_start(wcat[:], wcat_d[:])
            for rep in range(repeat):
                if rep:
                    tc.strict_bb_all_engine_barrier()
                for i in range(nt):
                    xt = pool.tile([P, P], f16, tag="xt")
                    nc.sync.dma_start(xt[:], xts_d[:, i * P : (i + 1) * P])
                    hp = psum.tile([P, wcols], f32, tag="hp")
                    nc.tensor.matmul(hp[:], lhsT=xt[:], rhs=wcat[:], start=True, stop=True)
                    hf = pool.tile([P, wcols], f16, tag="hf")
                    nc.scalar.copy(hf[:], hp[:])
                    nc.sync.dma_start(out_d[i * P : (i + 1) * P, :], hf[:])
    nc.compile()
    return nc


def build_gat_layer(nper, Ts, bandof, wcols, is_last, repeat=1, trn_type="TRN2", cfg=None):
    """Launch B/C: banded edge aggregation + node phase for one GAT layer.

    Ts: per-block tile counts (static, same all cores). bandof: per-block
    list of per-tile band indices. Inputs eah/eas/ead/m2 are slot-major
    per-partition contiguous fp16.
    """
    cfg = dict(CONFIG if cfg is None else cfg)
    nblocks = nper // P
    assert len(Ts) == nblocks
    TOT = int(sum(Ts))
    offs = np.concatenate([[0], np.cumsum(Ts)]).astype(int)
    zdt = f32 if is_last else f16

    nc = bacc.Bacc(trn_type, target_bir_lowering=False, debug=False, num_devices=NCORES)
    eah_d = nc.dram_tensor("eah", [P, TOT * D], f16, kind="ExternalInput")
    eas_d = nc.dram_tensor("eas", [P, TOT * HEADS], f16, kind="ExternalInput")
    ead_d = nc.dram_tensor("ead", [P, TOT * HEADS], f16, kind="ExternalInput")
    m2_d = nc.dram_tensor("m2", [P, TOT * 2], f16, kind="ExternalInput")
    wnext_d = nc.dram_tensor("wnext", [D, wcols], f16, kind="ExternalInput")
    brep_d = nc.dram_tensor("brep", [P, D], f16, kind="ExternalInput")
    bout_d = nc.dram_tensor("bout", [P, wcols], f32, kind="ExternalInput")
    iota_d = nc.dram_tensor("iota", [P, W], f16, kind="ExternalInput")
    ident_d = nc.dram_tensor("ident", [P, P], f16, kind="ExternalInput")
    out_d = nc.dram_tensor("zrows", [nper, wcols], zdt, kind="ExternalOutput")
    dbg = bool(cfg.get("debug"))
    if dbg:
        dbg_ew = nc.dram_tensor("dbg_ew", [P, TOT * HEADS], f16, kind="ExternalOutput")
        dbg_w2 = nc.dram_tensor("dbg_w2", [P, TOT * HEADS * 2], f16, kind="ExternalOutput")
        dbg_gw = nc.dram_tensor("dbg_gw", [P, TOT * RW], f16, kind="ExternalOutput")
        dbg_a01 = nc.dram_tensor("dbg_a01", [P, TOT * W], f16, kind="ExternalOutput")
        dbg_u = nc.dram_tensor("dbg_u", [P, nblocks * RW], f32, kind="ExternalOutput")
        dbg_h = nc.dram_tensor("dbg_h", [P, nblocks * D], f16, kind="ExternalOutput")
        dbg_ht = nc.dram_tensor("dbg_ht", [P, nblocks * P], f16, kind="ExternalOutput")

    with tile.TileContext(nc) as tc:
        with (
            tc.tile_pool(name="const", bufs=1) as cpool,
            tc.tile_pool(name="work", bufs=int(cfg.get("wbufs", 3))) as pool,
            tc.tile_pool(name="gath", bufs=int(cfg.get("gbufs", 3))) as gpool,
            tc.tile_pool(name="psum", bufs=int(cfg.get("zbufs", 2)), space="PSUM") as psum,
            tc.tile_pool(name="psU", bufs=int(cfg.get("ubufs", 2)), space="PSUM") as psU,
        ):
            wnext = cpool.tile([D, wcols], f16, tag="wnext")
            nc.sync.dma_start(wnext[:], wnext_d[:])
            brep = cpool.tile([P, D], f16, tag="brep")
            nc.sync.dma_start(brep[:], brep_d[:])
            bout = cpool.tile([P, wcols], f32, tag="bout")
            nc.sync.dma_start(bout[:], bout_d[:])
            iota = cpool.tile([P, W], f16, tag="iota")
            nc.sync.dma_start(iota[:], iota_d[:])
            ident = cpool.tile([P, P], f16, tag="ident")
            nc.sync.dma_start(ident[:], ident_d[:])
            m2all = cpool.tile([P, TOT * 2], f16, tag="m2all")
            nc.sync.dma_start(m2all[:], m2_d[:])
            ebias = cpool.tile([P, 1], f32, tag="ebias")
            nc.vector.memset(ebias[:], EXP_BIAS)

            for rep in range(repeat):
                if rep:
                    tc.strict_bb_all_engine_barrier()
                for b in range(nblocks):
                    T = int(Ts[b])
                    if T == 0:
                        continue
                    off = int(offs[b])
                    bands = bandof[b]

                    Gh = gpool.tile([P, T * D], f16, tag="Gh")
                    nc.sync.dma_start(Gh[:], eah_d[:, off * D : (off + T) * D])
                    Ga = pool.tile([P, T * HEADS], f16, tag="Ga")
                    nc.sync.dma_start(Ga[:], eas_d[:, off * HEADS : (off + T) * HEADS])
                    Gd = pool.tile([P, T * HEADS], f16, tag="Gd")
                    nc.sync.dma_start(Gd[:], ead_d[:, off * HEADS : (off + T) * HEADS])

                    # e = a_src + a_dst ; leaky
                    ew = pool.tile([P, T * HEADS], f16, tag="ew")
                    ew_eng = nc.gpsimd if cfg["ew_eng"] == "gp" else nc.vector
                    ew_eng.tensor_tensor(out=ew[:], in0=Ga[:], in1=Gd[:], op=mybir.AluOpType.add)
                    lrelu = cfg["lrelu"]
                    if lrelu in ("act", "act8"):
                        nc.scalar.activation(ew[:], ew[:], mybir.ActivationFunctionType.Lrelu,
                                             alpha=NEG_SLOPE * (4.0 if lrelu == "act8" else 1.0))
                    elif lrelu in ("prelu", "prelu8"):
                        nc.scalar.activation(ew[:], ew[:], mybir.ActivationFunctionType.Prelu,
                                             alpha=NEG_SLOPE * (4.0 if lrelu == "prelu8" else 1.0))
                    else:
                        # "mixed": scale on GPSIMD, max on DVE (Pool lacks max)
                        lr_eng = nc.gpsimd if lrelu == "mixed" else nc.vector
                        lk = pool.tile([P, T * HEADS], f16, tag="lk")
                        lr_eng.tensor_scalar(lk[:], ew[:], NEG_SLOPE, None, mybir.AluOpType.mult)
                        nc.vector.tensor_tensor(out=ew[:], in0=ew[:], in1=lk[:], op=mybir.AluOpType.max)

                    # w2[p, t, head, 0:2] = exp(e - 4)  (pair-replicated)
                    w2 = pool.tile([P, T * HEADS * 2], f16, tag="w2")
                    ew_b = _free_dims(ew[:], [[HEADS, T], [1, HEADS], [0, 2]])
                    w2_o = _free_dims(w2[:], [[2 * HEADS, T], [2, HEADS], [1, 2]])
                    nc.scalar.activation(w2_o, ew_b, mybir.ActivationFunctionType.Exp,
                                         bias=ebias[:])

                    # GW = [h*w | w] per tile (RW=132 cols)
                    GW = gpool.tile([P, T * RW], f16, tag="GW")
                    gw3 = GW[:].rearrange("p (t c) -> p t c", c=RW)
                    # w cols: exp again, strided out
                    nc.scalar.activation(
                        gw3[:, :, D:RW],
                        _free_dims(ew[:], [[HEADS, T], [1, HEADS]]),
                        mybir.ActivationFunctionType.Exp,
                        bias=ebias[:],
                    )
                    # h*w: DVE 2x (all APs innermost step-1 fp16); optionally a
                    # leading fraction of tiles goes to GPSIMD instead
                    kgp = int(round(T * float(cfg.get("gw_gp_frac", 0.0))))
                    PAIR_REP = D // HEADS // 2
                    if kgp > 0:
                        nc.gpsimd.tensor_tensor(
                            out=gw3[:, 0:kgp, 0:D],
                            in0=_free_dims(Gh[:], [[D, kgp], [1, D]]),
                            in1=_free_dims(w2[:], [[2 * HEADS, kgp], [2, HEADS], [0, PAIR_REP], [1, 2]]),
                            op=mybir.AluOpType.mult,
                        )
                    if kgp < T:
                        w2k = w2[:, kgp * 2 * HEADS : T * 2 * HEADS]
                        ghk = Gh[:, kgp * D : T * D]
                        nc.vector.tensor_tensor(
                            out=gw3[:, kgp:T, 0:D],
                            in0=_free_dims(ghk, [[D, T - kgp], [1, D]]),
                            in1=_free_dims(w2k, [[2 * HEADS, T - kgp], [2, HEADS], [0, PAIR_REP], [1, 2]]),
                            op=mybir.AluOpType.mult,
                        )

                    # banded one-hot A01[p, t, d] = (m2[p,t] == d), d in [0,32)
                    A01 = gpool.tile([P, T * W], f16, tag="A01")
                    m2_in = _free_dims(m2all[:], [[2, T], [0, W // 2], [1, 2]])
                    m2_in = dataclasses.replace(m2_in, offset=m2_in.offset + 2 * off)
                    iota_in = _free_dims(iota[:], [[0, T], [1, W]])
                    nc.vector.tensor_tensor(
                        out=A01[:], in0=m2_in, in1=iota_in, op=mybir.AluOpType.is_equal
                    )

                    # aggregate: U[band*W:(band+1)*W] += A01_t.T @ [h*w | w]_t
                    a3 = A01[:].rearrange("p (t d) -> p t d", d=W)
                    U = psU.tile([P, RW], f32, tag="U")
                    if cfg["start_mode"] == "t0":
                        starts = [t == 0 for t in range(T)]
                    else:
                        seen = set()
                        starts = []
                        for t in range(T):
                            starts.append(bands[t] not in seen)
                            seen.add(bands[t])
                    for t in range(T):
                        Bt = int(bands[t]) * W
                        nc.tensor.matmul(
                            U[Bt : Bt + W, :],
                            lhsT=a3[:, t, :],
                            rhs=gw3[:, t, :],
                            start=starts[t],
                            stop=(t == T - 1),
                            skip_group_check=True,
                        )

                    if dbg:
                        nc.sync.dma_start(dbg_ew[:, off * HEADS : (off + T) * HEADS], ew[:])
                        nc.sync.dma_start(dbg_w2[:, off * HEADS * 2 : (off + T) * HEADS * 2], w2[:])
                        nc.sync.dma_start(dbg_gw[:, off * RW : (off + T) * RW], GW[:])
                        nc.sync.dma_start(dbg_a01[:, off * W : (off + T) * W], A01[:])
                        ucp = pool.tile([P, RW], f32, tag="ucp")
                        nc.scalar.copy(ucp[:], U[:, :])
                        nc.sync.dma_start(dbg_u[:, b * RW : (b + 1) * RW], ucp[:])

                    # node phase: h = relu(U_h / s + b)
                    h = pool.tile([P, D], f16, tag="h")
                    if cfg["div"] == "div":
                        s_sb = pool.tile([P, HEADS], f32, tag="s_sb")
                        nc.scalar.copy(s_sb[:], U[:, D:RW])
                        s_b = _free_dims(s_sb[:], [[1, HEADS], [0, D // HEADS]])
                        nc.vector.tensor_tensor(
                            out=h[:], in0=U[:, 0:D], in1=s_b, op=mybir.AluOpType.divide
                        )
                    else:
                        rcp = pool.tile([P, HEADS], f32, tag="rcp")
                        if cfg.get("noeps"):
                            nc.vector.reciprocal(rcp[:], U[:, D:RW])
                        else:
                            s_eps = pool.tile([P, HEADS], f32, tag="s_eps")
                            nc.vector.tensor_scalar_add(s_eps[:], U[:, D:RW], 1e-16)
                            nc.vector.reciprocal(rcp[:], s_eps[:])
                        rcp_b = _free_dims(rcp[:], [[1, HEADS], [0, D // HEADS]])
                        nc.vector.tensor_tensor(
                            out=h[:], in0=U[:, 0:D], in1=rcp_b, op=mybir.AluOpType.mult
                        )
                    hadd_eng = nc.gpsimd if cfg["hadd_eng"] == "gp" else nc.vector
                    hadd_eng.tensor_tensor(out=h[:], in0=h[:], in1=brep[:], op=mybir.AluOpType.add)
                    nc.scalar.activation(h[:], h[:], mybir.ActivationFunctionType.Relu)

                    # project: zrows = hT.T @ wnext (+ bout for last layer)
                    hT = pool.tile([P, P], f16, tag="hT")
                    if cfg["trans"] == "dma":
                        nc.sync.dma_start_transpose(hT[:], h[:])
                    else:
                        hTp = psum.tile([P, P], f16, tag="hTp")
                        nc.tensor.transpose(hTp[:], h[:], ident[:])
                        nc.scalar.copy(hT[:], hTp[:])
                    if dbg:
                        nc.sync.dma_start(dbg_h[:, b * D : (b + 1) * D], h[:])
                        nc.sync.dma_start(dbg_ht[:, b * P : (b + 1) * P], hT[:])
                    zp = psum.tile([P, wcols], f32, tag="zp")
                    nc.tensor.matmul(zp[:], lhsT=hT[:], rhs=wnext[:], start=True, stop=True)
                    z = pool.tile([P, wcols], zdt, tag="z")
                    if is_last:
                        nc.vector.tensor_tensor(out=z[:], in0=zp[:], in1=bout[:], op=mybir.AluOpType.add)
                    else:
                        nc.scalar.copy(z[:], zp[:])
                    nc.sync.dma_start(out_d[b * P : (b + 1) * P, :], z[:])

    nc.compile()
    return nc


def prep_edges(edge_index, n, ncores):
    """Band-aligned slot layout. Returns dict with static structure
    (Ts, bandof, offs) and per-core slot data (ss, ds, m2)."""
    nper = -(-n // (ncores * P)) * P
    npad = nper * ncores
    nblocks = nper // P

    e0 = np.asarray(edge_index[0], dtype=np.int64)
    e1 = np.asarray(edge_index[1], dtype=np.int64)
    loops = np.arange(n, dtype=np.int64)
    src = np.concatenate([e0, loops])
    dst = np.concatenate([e1, loops])
    order = np.argsort(dst, kind="stable")
    srcs = src[order]
    dsts = dst[order]

    # per (core, block, band) counts via searchsorted on band boundaries
    bounds = np.searchsorted(dsts, np.arange(0, npad + W, W))  # [npad/W + 1]
    cnt = (bounds[1:] - bounds[:-1]).reshape(ncores, nblocks, NBAND)
    tiles = -(-cnt.max(axis=0) // P)  # [nblocks, NBAND] (0 where empty)
    Ts = tiles.sum(axis=1)  # [nblocks]
    TOT = int(Ts.sum())
    offs = np.concatenate([[0], np.cumsum(Ts)]).astype(int)
    bandof = [
        np.repeat(np.arange(NBAND), tiles[b]).astype(int).tolist()
        for b in range(nblocks)
    ]
    # tile start (within block) of each band
    tbase = np.concatenate(
        [np.zeros((nblocks, 1), int), np.cumsum(tiles, axis=1)[:, :-1]], axis=1
    )

    ss_l, ds_l, m2_l = [], [], []
    for c in range(ncores):
        ss = np.zeros(TOT * P, dtype=np.int64)
        ds_ = np.zeros(TOT * P, dtype=np.int64)
        meta = np.full((P, TOT), 300.0, dtype=np.float16)
        for b in range(nblocks):
            for j in range(NBAND):
                g = (c * nblocks + b) * NBAND + j
                lo, hi = int(bounds[g]), int(bounds[g + 1])
                cntb = hi - lo
                if cntb == 0:
                    continue
                t0 = int(offs[b] + tbase[b, j])
                k = np.arange(cntb)
                slot = (t0 + k // P) * P + (k % P)
                ss[slot] = srcs[lo:hi]
                ds_[slot] = dsts[lo:hi]
                loc = (dsts[lo:hi] - (c * nper + b * P + j * W)).astype(np.float16)
                meta[k % P, t0 + k // P] = loc
        m2 = np.repeat(meta, 2, axis=1)  # [P, TOT*2] pair-replicated
        ss_l.append(ss)
        ds_l.append(ds_)
        m2_l.append(np.ascontiguousarray(m2))
    return {
        "Ts": Ts, "bandof": bandof, "offs": offs, "TOT": TOT, "npad": npad,
        "ss": ss_l, "ds": ds_l, "m2": m2_l,
    }


def expand_rows(nrows_full, prep):
    """Per-core (eah, eas, ead) in slot-major per-partition-contiguous
    fp16 layout. nrows_full: [npad, ROW] fp16."""
    TOT = prep["TOT"]
    outs = []
    for ss, ds_ in zip(prep["ss"], prep["ds"]):
        g = nrows_full[ss]  # [TOT*P, ROW]
        eah = np.ascontiguousarray(
            g[:, 0:D].reshape(TOT, P, D).transpose(1, 0, 2).reshape(P, TOT * D)
        )
        eas = np.ascontiguousarray(
            g[:, D : D + HEADS].reshape(TOT, P, HEADS).transpose(1, 0, 2).reshape(P, TOT * HEADS)
        )
        gd = nrows_full[ds_, D + HEADS : ROW]
        ead = np.ascontiguousarray(
            gd.reshape(TOT, P, HEADS).transpose(1, 0, 2).reshape(P, TOT * HEADS)
        )
        outs.append((eah, eas, ead))
    return outs


def amat(att):
    A = np.zeros((D, HEADS), dtype=np.float32)
    att = np.asarray(att, dtype=np.float32)
    for h in range(HEADS):
        A[h * (D // HEADS) : (h + 1) * (D // HEADS), h] = att[h]
    return A


_cache = {}


def run_gat(x, edge_index, W1, att_src1, att_dst1, b1, W2, att_src2, att_dst2, b2,
            Wc, bc, n=None, ncores=NCORES, repeat=1, cfg=None):
    global LAST_INFO
    x = np.asarray(x, dtype=np.float32)
    if n is None:
        n = int(x.shape[0])

    t0 = time.time()
    prep = prep_edges(edge_index, n, ncores)
    npad = prep["npad"]
    nper = npad // ncores
    cfg = dict(CONFIG if cfg is None else cfg)
    key = (npad, tuple(prep["Ts"]), ncores, repeat, tuple(sorted(cfg.items())))
    t1 = time.time()
    if key in _cache:
        ncA, ncB, ncC = _cache[key]
    else:
        ncA = build_node_transform(nper, ROW)
        ncB = build_gat_layer(nper, prep["Ts"], prep["bandof"], ROW, is_last=False,
                              repeat=repeat, cfg=cfg)
        ncC = build_gat_layer(nper, prep["Ts"], prep["bandof"], C, is_last=True,
                              repeat=repeat, cfg=cfg)
        _cache[key] = (ncA, ncB, ncC)
    t2 = time.time()

    W1 = np.asarray(W1, dtype=np.float32)
    W2 = np.asarray(W2, dtype=np.float32)
    Wc = np.asarray(Wc, dtype=np.float32)
    w1cat = np.concatenate([W1, W1 @ amat(att_src1), W1 @ amat(att_dst1)], axis=1).astype(np.float16)
    w2cat = np.concatenate([W2, W2 @ amat(att_src2), W2 @ amat(att_dst2)], axis=1).astype(np.float16)
    b1r = np.tile(np.asarray(b1, np.float16)[None, :], (P, 1))
    b2r = np.tile(np.asarray(b2, np.float16)[None, :], (P, 1))
    bc = np.asarray(bc, dtype=np.float32)
    iota = np.tile(np.arange(W, dtype=np.float16), (P, 1))
    ident = np.eye(P, dtype=np.float16)
    zero_bout = np.zeros((P, ROW), np.float32)

    xp = np.zeros((npad, D), dtype=np.float16)
    xp[:n] = x.astype(np.float16)

    # Launch A
    mapsA = [
        {"xts": np.ascontiguousarray(xp[c * nper : (c + 1) * nper].T), "wcat": w1cat}
        for c in range(ncores)
    ]
    resA = run_bass_kernel_spmd(ncA, mapsA, list(range(ncores)))
    nrows_full = np.concatenate([resA.results[c]["nrows"] for c in range(ncores)], axis=0)
    t3 = time.time()

    # Expansion 1 + Launch B
    ea1 = expand_rows(nrows_full, prep)
    mapsB = [
        {"eah": ea1[c][0], "eas": ea1[c][1], "ead": ea1[c][2], "m2": prep["m2"][c],
         "wnext": w2cat, "brep": b1r, "bout": zero_bout, "iota": iota, "ident": ident}
        for c in range(ncores)
    ]
    resB = run_bass_kernel_spmd(ncB, mapsB, list(range(ncores)))
    zrows_full = np.concatenate([resB.results[c]["zrows"] for c in range(ncores)], axis=0)
    t4 = time.time()

    # Expansion 2 + Launch C
    ea2 = expand_rows(zrows_full, prep)
    mapsC = [
        {"eah": ea2[c][0], "eas": ea2[c][1], "ead": ea2[c][2], "m2": prep["m2"][c],
         "wnext": Wc.astype(np.float16), "brep": b2r,
         "bout": np.tile(bc[None, :], (P, 1)), "iota": iota, "ident": ident}
        for c in range(ncores)
    ]
    resC = run_bass_kernel_spmd(ncC, mapsC, list(range(ncores)))
    out = np.concatenate([resC.results[c]["zrows"] for c in range(ncores)], axis=0)[:n]
    t5 = time.time()

    LAST_INFO = {
        "prep_s": t1 - t0, "build_s": t2 - t1, "launchA_s": t3 - t2,
        "launchB_s": t4 - t3, "launchC_s": t5 - t4,
        "ncs": (ncA, ncB, ncC),
        "maps": (mapsA, mapsB, mapsC),
        "prep": prep,
    }
    print(
        f"[kernel] prep={t1 - t0:.2f}s build={t2 - t1:.2f}s A={t3 - t2:.2f}s "
        f"B={t4 - t3:.2f}s C={t5 - t4:.2f}s",
        file=sys.stderr,
    )
    return out.astype(np.float32)


def kernel(x, edge_index, W1, att_src1, att_dst1, b1, W2, att_src2, att_dst2, b2, Wc, bc):
    return run_gat(x, edge_index, W1, att_src1, att_dst1, b1,
                   W2, att_src2, att_dst2, b2, Wc, bc)
